# revision 1
# baseline (speedup 1.0000x reference)
"""Trainium2 Bass kernel for nn_CrossAttention (B=2, N=2048, M=256, C=1024, H=16).

Sharding: 8 cores = 2 batches x 4 head-groups (4 heads each).
Each core computes its heads' QKV/KV projections, qk-RMSNorm, attention and a
partial output projection over its 256 channels; the host sums the 4 partials
per batch (the all-reduce) and adds proj_b.
"""

import sys

sys.path.insert(0, "/opt/trn_rl_repo")

import numpy as np  # noqa: E402

import concourse.bass as bass  # noqa: E402
import concourse.tile as tile  # noqa: E402
from concourse import bacc, mybir  # noqa: E402
from concourse.bass_utils import run_bass_kernel_spmd  # noqa: E402

F32 = mybir.dt.float32
R32 = mybir.dt.float32r
AF = mybir.ActivationFunctionType
MUL = mybir.AluOpType.mult

H = 16
B = 2
N = 2048          # image tokens
M = 256           # text tokens
C = 1024
HD = 64           # head dim
EPS = 1e-6
S = N + M         # 2304 kv length
HPC = 4           # heads per core
NT = 512          # query tile
SCALE = HD ** -0.5





_TCNT = [0]


def T(pool, shape, tag, bufs=None, dt=F32):
    _TCNT[0] += 1
    kw = dict(tag=tag, name=f"{tag}_{_TCNT[0]}")
    if bufs is not None:
        kw["bufs"] = bufs
    return pool.tile(shape, dt, **kw)


def build_program(loop_iters=None):
    nc = bacc.Bacc("TRN2", target_bir_lowering=False, debug=False)

    xT = nc.dram_tensor("xT", [C, N], R32, kind="ExternalInput").ap()
    yT = nc.dram_tensor("yT", [C, M], R32, kind="ExternalInput").ap()
    wqkvT = nc.dram_tensor("wqkvT", [C, 2 * HPC * HD], R32, kind="ExternalInput").ap()
    bqkv = nc.dram_tensor("bqkv", [128, 4], F32, kind="ExternalInput").ap()
    wkvT = nc.dram_tensor("wkvT", [C, HPC * HD], R32, kind="ExternalInput").ap()
    wvxT = nc.dram_tensor("wvxT", [C, HPC * HD], R32, kind="ExternalInput").ap()
    wvyT = nc.dram_tensor("wvyT", [C, HPC * HD], R32, kind="ExternalInput").ap()
    bvx = nc.dram_tensor("bvx", [1, HPC * HD], R32, kind="ExternalInput").ap()
    bvy = nc.dram_tensor("bvy", [1, HPC * HD], R32, kind="ExternalInput").ap()
    ones1r = nc.dram_tensor("ones1r", [1, 128], R32, kind="ExternalInput").ap()
    bkv = nc.dram_tensor("bkv", [128, 2], F32, kind="ExternalInput").ap()
    wprojT = nc.dram_tensor("wprojT", [HPC * HD, C], R32, kind="ExternalInput").ap()
    qknw = nc.dram_tensor("qknw", [128, 2], F32, kind="ExternalInput").ap()
    onesb = nc.dram_tensor("onesb", [128, 2], R32, kind="ExternalInput").ap()
    ones2 = nc.dram_tensor("ones2", [2, 128], R32, kind="ExternalInput").ap()
    sel64 = nc.dram_tensor("sel64", [65, 64], R32, kind="ExternalInput").ap()
    vones = nc.dram_tensor("vones", [128, 18, 1], R32, kind="ExternalInput").ap()
    outT = nc.dram_tensor("outT", [C, N], F32, kind="ExternalOutput").ap()

    with tile.TileContext(nc) as tc:
        with (
            tc.tile_pool(name="const", bufs=1) as const,
            tc.tile_pool(name="sing", bufs=1) as sing,
        ):
            yT_sb = T(const, [128, 8, M], "yT", dt=R32)
            nc.sync.dma_start(yT_sb, yT.rearrange("(o p) f -> p o f", p=128))
            wkv_sb = T(const, [128, 8, HPC * HD], "wkv", dt=R32)
            nc.sync.dma_start(wkv_sb, wkvT.rearrange("(o p) f -> p o f", p=128))
            wvy_sb = T(const, [128, 8, HPC * HD], "wvy", dt=R32)
            nc.sync.dma_start(wvy_sb, wvyT.rearrange("(o p) f -> p o f", p=128))
            wvx_sb = T(const, [128, 8, HPC * HD], "wvx", dt=R32)
            nc.sync.dma_start(wvx_sb, wvxT.rearrange("(o p) f -> p o f", p=128))
            bvx_sb = T(const, [1, HPC * HD], "bvx", dt=R32)
            nc.sync.dma_start(bvx_sb, bvx)
            bvy_sb = T(const, [1, HPC * HD], "bvy", dt=R32)
            nc.sync.dma_start(bvy_sb, bvy)
            ones1_sb = T(const, [1, 128], "ones1r", dt=R32)
            nc.sync.dma_start(ones1_sb, ones1r)
            wqkv_sb = T(const, [128, 8, 2 * HPC * HD], "wqkv", dt=R32)
            wqkv_r = wqkvT.rearrange("(o p) f -> p o f", p=128)
            for cc in range(8):
                nc.sync.dma_start(wqkv_sb[:, cc], wqkv_r[:, cc])
            wproj_sb = T(const, [64, 4, C], "wproj", dt=R32)
            nc.sync.dma_start(wproj_sb, wprojT.rearrange("(c p) o -> p c o", p=64))
            bqkv_sb = T(const, [128, 4], "bqkv")
            nc.sync.dma_start(bqkv_sb, bqkv)
            bkv_sb = T(const, [128, 2], "bkv")
            nc.sync.dma_start(bkv_sb, bkv)
            qknw_sb = T(const, [128, 2], "qknw")
            nc.sync.dma_start(qknw_sb, qknw)
            onesb_sb = T(const, [128, 2], "onesb", dt=R32)
            nc.sync.dma_start(onesb_sb, onesb)
            ones2_sb = T(const, [2, 128], "ones2", dt=R32)
            nc.sync.dma_start(ones2_sb, ones2)
            sel_sb = T(const, [65, 64], "sel64", dt=R32)
            nc.sync.dma_start(sel_sb, sel64)
            eps_sb = T(const, [128, 1], "epsc")
            nc.vector.memset(eps_sb, float(EPS))
            zero_sb = T(const, [128, 1], "zeroc")
            nc.vector.memset(zero_sb, 0.0)

            # persistent activations: channel-on-partition layouts
            qT = T(sing, [128, 2, N], "qT", dt=R32)       # [2 heads x 64d, hp, n]
            kT = T(sing, [128, 2, S], "kT", dt=R32)
            vS = T(sing, [128, 18, HPC * 65], "vS", dt=R32)  # [s%128, s//128, h*65+(d|one)]
            for h in range(HPC):
                nc.sync.dma_start(vS[:, :, 65 * h + 64 : 65 * h + 65], vones)

            def norm_chunk(pool_ps, pool_wk, psum, bias_ap, w_col, dest):
                """dest = (psum + bias) * rsqrt(mean_d((psum+bias)^2)+eps) * w"""
                nsz = psum.shape[-1]
                tb = T(pool_wk, [128, NT], "w")[:, :nsz]
                nc.vector.tensor_scalar_add(tb, psum, bias_ap)
                sq = T(pool_wk, [128, NT], "w", dt=R32)[:, :nsz]
                nc.vector.tensor_mul(sq, tb, tb)
                ssp = T(pool_ps, [2, NT], "paux", bufs=3)[:, :nsz]
                nc.tensor.matmul(ssp, onesb_sb, sq, start=True, stop=True)
                lnv = T(pool_wk, [2, NT], "w2", bufs=8)[:, :nsz]
                nc.scalar.activation(
                    lnv, ssp, AF.Ln, bias=eps_sb[0:2], scale=1.0 / HD
                )
                rmsv = T(pool_wk, [2, NT], "w2", bufs=8, dt=R32)[:, :nsz]
                nc.scalar.activation(rmsv, lnv, AF.Exp, bias=zero_sb[0:2], scale=-0.5)
                rbc = T(pool_ps, [128, NT], "paux", bufs=3)[:, :nsz]
                nc.tensor.matmul(rbc, ones2_sb, rmsv, start=True, stop=True)
                t2 = T(pool_wk, [128, NT], "w")[:, :nsz]
                nc.vector.tensor_mul(t2, tb, rbc)
                nc.vector.tensor_scalar_mul(dest, t2, w_col)

            def v_proj(pool_ps, src_sb, t, w_sb, b_sb, j):
                """vS[:, j] = (src.T @ wv + bv) directly in [s, d] layout."""
                pv = T(pool_ps, [128, HPC * HD], "pmain", bufs=4)
                for cc in range(8):
                    nc.tensor.matmul(
                        pv,
                        src_sb[:, cc, t * 128 : (t + 1) * 128],
                        w_sb[:, cc, :],
                        start=(cc == 0),
                        stop=False,
                    )
                nc.tensor.matmul(pv, ones1_sb, b_sb, start=False, stop=True)
                dst = vS[:, j, :].rearrange("p (a b) -> p a b", b=65)[:, :, 0:64]
                nc.vector.tensor_copy(
                    out=dst, in_=pv.rearrange("p (a b) -> p a b", b=64)
                )

            # ---- phase 1: KV projection of y (text tokens -> kv rows 2048..2303)
            import contextlib
            with contextlib.ExitStack() as _les:
                if loop_iters is not None:
                    _les.enter_context(tc.For_i(0, loop_iters, 1))
                with (
                    tc.tile_pool(name="pp12", bufs=3, space="PSUM") as pp12,
                    tc.tile_pool(name="wk", bufs=12) as wk,
                ):
                    for mc in range(2):  # [k01, k23]
                        ps = T(pp12, [128, NT], "pmain", bufs=4)[:, :M]
                        for cc in range(8):
                            nc.tensor.matmul(
                                ps,
                                wkv_sb[:, cc, mc * 128 : (mc + 1) * 128],
                                yT_sb[:, cc, :],
                                start=(cc == 0),
                                stop=(cc == 7),
                            )
                        norm_chunk(
                            pp12, wk, ps, bkv_sb[:, mc : mc + 1],
                            qknw_sb[:, 1:2], kT[:, mc, N : N + M],
                        )
                    for t in range(2):
                        v_proj(pp12, yT_sb, t, wvy_sb, bvy_sb, 16 + t)

                    # ---- phase 2: QKV projection of x
                    with tc.tile_pool(name="xin", bufs=2) as xin:
                        for nt in range(N // NT):
                            nsl = slice(nt * NT, (nt + 1) * NT)
                            xc = T(xin, [128, 8, NT], "xc", dt=R32)
                            nc.sync.dma_start(
                                xc, xT.rearrange("(o p) f -> p o f", p=128)[:, :, nsl]
                            )
                            for mc in range(4):  # [q01,q23,k01,k23]
                                ps = T(pp12, [128, NT], "pmain", bufs=4)
                                for cc in range(8):
                                    nc.tensor.matmul(
                                        ps,
                                        wqkv_sb[:, cc, mc * 128 : (mc + 1) * 128],
                                        xc[:, cc, :],
                                        start=(cc == 0),
                                        stop=(cc == 7),
                                    )
                                bias_ap = bqkv_sb[:, mc : mc + 1]
                                if mc < 2:
                                    norm_chunk(pp12, wk, ps, bias_ap,
                                               qknw_sb[:, 0:1], qT[:, mc, nsl])
                                else:
                                    norm_chunk(pp12, wk, ps, bias_ap,
                                               qknw_sb[:, 1:2], kT[:, mc - 2, nsl])
                            for t in range(4):
                                v_proj(pp12, xc, t, wvx_sb, bvx_sb, nt * 4 + t)

                # ---- phase 3+4: attention + output projection, per query tile
                with (
                    tc.tile_pool(name="pa", bufs=2, space="PSUM") as pa,
                    tc.tile_pool(name="atp", bufs=3) as atp,
                    tc.tile_pool(name="asp", bufs=3) as asp,
                    tc.tile_pool(name="outp", bufs=2) as outp,
                    tc.tile_pool(name="osp", bufs=2) as osp,
                ):
                    for nt in range(N // NT):
                        nsl = slice(nt * NT, (nt + 1) * NT)
                        ot = T(outp, [64, HPC, NT], "ot", dt=R32)
                        for hp in range(2):
                            avs_list = []
                            av_list = [
                                T(pa, [128, NT], "avpo", bufs=2)[:65],
                                T(pa, [128, NT], "avpo", bufs=2)[:65],
                            ]
                            for jg in range(6):
                                j0 = 3 * jg
                                for idx in range(2):
                                    h = 2 * hp + idx
                                    prt = slice(64 * idx, 64 * idx + 64)
                                    tp = (64 * idx, 0)
                                    pl = T(pa, [128, 3 * NT], "big", bufs=2)
                                    rhsQ = qT[prt, hp, nsl]
                                    for u in range(3):
                                        nc.tensor.matmul(
                                            pl[:, u * NT : (u + 1) * NT],
                                            kT[prt, hp,
                                               (j0 + u) * 128 : (j0 + u + 1) * 128],
                                            rhsQ, start=True, stop=True,
                                            tile_position=tp,
                                        )
                                    at = T(atp, [128, 3 * NT], "at", dt=R32)
                                    nc.scalar.activation(
                                        at, pl, AF.Exp, bias=zero_sb[:], scale=SCALE
                                    )
                                    av = av_list[idx]
                                    for u in range(3):
                                        nc.tensor.matmul(
                                            av,
                                            vS[:, j0 + u, 65 * h : 65 * h + 65],
                                            at[:, u * NT : (u + 1) * NT],
                                            start=(j0 + u == 0),
                                            stop=(j0 + u == 17),
                                        )
                            for idx in range(2):
                                h = 2 * hp + idx
                                avs = T(asp, [65, NT], "avs", dt=R32)
                                nc.vector.tensor_copy(avs, av_list[idx])
                                dbc = T(pa, [64, NT], "big", bufs=2)
                                nc.tensor.matmul(
                                    dbc, sel_sb, avs, start=True, stop=True
                                )
                                rbc = T(asp, [64, NT], "rbc")
                                nc.vector.reciprocal(rbc, dbc)
                                nc.vector.tensor_mul(ot[:, h, :], avs[0:64, :], rbc)

                        for oc in range(8):
                            po = T(pa, [128, NT], "avpo", bufs=2)
                            for cc in range(4):
                                nc.tensor.matmul(
                                    po,
                                    wproj_sb[:, cc, oc * 128 : (oc + 1) * 128],
                                    ot[:, cc, :],
                                    start=(cc == 0), stop=(cc == 3),
                                )
                            ob = T(osp, [128, NT], "ob")
                            nc.vector.tensor_copy(ob, po)
                            nc.sync.dma_start(
                                outT.rearrange("(o p) f -> p o f", p=128)[:, oc, nsl],
                                ob,
                            )
    _orig = bacc.get_activation_tables

    def _tables(arch):
        t = _orig(arch)
        return {
            name: (set() if name in ("exp_and_others", "natural_log",
                                     "exp_and_friends") else fns)
            for name, fns in t.items()
        }

    bacc.get_activation_tables = _tables
    try:
        nc.compile()
    finally:
        bacc.get_activation_tables = _orig
    return nc


_PROGRAM = None


def _get_program():
    global _PROGRAM
    if _PROGRAM is None:
        _PROGRAM = build_program()
    return _PROGRAM


def _make_in_maps(x, y, qkv_w, qkv_b, kv_w, kv_b, qn_w, kn_w, proj_w, proj_b):
    f = np.float32
    onesb = np.zeros((128, 2), f)
    onesb[0:64, 0] = 1.0
    onesb[64:128, 1] = 1.0
    ones2 = np.zeros((2, 128), f)
    ones2[0, 0:64] = 1.0
    ones2[1, 64:128] = 1.0
    sel64 = np.zeros((65, 64), f)
    sel64[64, :] = 1.0
    qknw = np.stack([np.tile(qn_w, 2), np.tile(kn_w, 2)], axis=1).astype(f)

    in_maps = []
    for core in range(8):
        b, g = divmod(core, 4)
        qs = slice(g * 256, (g + 1) * 256)
        wqkv = np.concatenate([qkv_w[qs], qkv_w[1024:2048][qs]], axis=0)
        bq = np.concatenate([qkv_b[qs], qkv_b[1024:2048][qs]])
        wkv = kv_w[qs]
        bk = kv_b[qs]
        wvx = qkv_w[2048:3072][qs]
        bvxv = qkv_b[2048:3072][qs]
        wvy = kv_w[1024:2048][qs]
        bvyv = kv_b[1024:2048][qs]
        in_maps.append(
            {
                "xT": np.ascontiguousarray(x[b].T, f),
                "yT": np.ascontiguousarray(y[b].T, f),
                "wqkvT": np.ascontiguousarray(wqkv.T, f),
                "bqkv": np.ascontiguousarray(bq.reshape(4, 128).T, f),
                "wkvT": np.ascontiguousarray(wkv.T, f),
                "bkv": np.ascontiguousarray(bk.reshape(2, 128).T, f),
                "wvxT": np.ascontiguousarray(wvx.T, f),
                "bvx": np.ascontiguousarray(bvxv.reshape(1, 256), f),
                "wvyT": np.ascontiguousarray(wvy.T, f),
                "bvy": np.ascontiguousarray(bvyv.reshape(1, 256), f),
                "ones1r": np.ones((1, 128), f),
                "wprojT": np.ascontiguousarray(proj_w[:, qs].T, f),
                "qknw": qknw,
                "onesb": onesb,
                "ones2": ones2,
                "sel64": sel64,
                "vones": np.ones((128, 18, 1), f),
            }
        )
    return in_maps


def run_cores(inputs, trace=False, **kwargs):
    nc = _get_program()
    in_maps = _make_in_maps(**{k: np.asarray(v, np.float32) for k, v in inputs.items()})
    return run_bass_kernel_spmd(
        nc, in_maps, core_ids=list(range(8)), trace=trace, **kwargs
    )


def kernel(**inputs):
    proj_b = np.asarray(inputs["proj_b"], np.float32)
    res = run_cores(inputs).results
    out = np.zeros((B, N, C), np.float32)
    for core in range(8):
        b = core // 4
        out[b] += res[core]["outT"].T
    out += proj_b[None, None, :]
    return out



# revision 2
# speedup vs baseline: 3.1582x; 3.1582x over previous
"""Trainium2 Bass kernel for nn_CrossAttention (B=2, N=2048, M=256, C=1024, H=16).

Sharding: 8 cores = 2 batches x 4 head-groups (4 heads each). Each core
computes its heads' QKV/KV projections, qk-RMSNorm, attention and a partial
output projection over its 256 channels; the host sums the 4 partials per
batch (the all-reduce) and adds proj_b.

v2 design vs baseline:
- all big matmuls in bf16 (same PE rate as fp32r, half the SBUF/DMA)
- softmax exp on the Vector engine via Schraudolph bit-trick
  (bf16 = bitcast(int16(alpha*x + beta))) instead of ScalarE activation:
  ~3x faster per tile and frees the Scalar engine
- AV matmuls pack 2 heads via column tiling (64+64 of 128 cols)
- softmax denominators via 4-way col-tiled ones-matmuls into a shared bank
- output projection packs 2 head-pairs on the contraction dim (128 rows)
- PSUM->SBUF evacuations moved to the otherwise-idle Scalar engine
"""

import sys

sys.path.insert(0, "/opt/trn_rl_repo")

import numpy as np  # noqa: E402
import ml_dtypes  # noqa: E402

import concourse.bass as bass  # noqa: E402
import concourse.tile as tile  # noqa: E402
from concourse import bacc, mybir  # noqa: E402
from concourse.bass_utils import run_bass_kernel_spmd  # noqa: E402

F32 = mybir.dt.float32
R32 = mybir.dt.float32r
BF16 = mybir.dt.bfloat16
I16 = mybir.dt.int16
AF = mybir.ActivationFunctionType
MUL = mybir.AluOpType.mult
ADD = mybir.AluOpType.add

H = 16
B = 2
N = 2048          # image tokens
M = 256           # text tokens
C = 1024
HD = 64           # head dim
EPS = 1e-6
S = N + M         # 2304 kv length
NCH = S // 128    # 18 kv chunks
HPC = 4           # heads per core
NT = 512          # query tile
SCALE = HD ** -0.5
LN2 = float(np.log(2.0))
ALPHA = SCALE * 128.0 / LN2       # fold logit scale into the exp bit-trick
BETA = 127.0 * 128.0              # bf16 exponent bias in int16 units
SQ05 = float(np.sqrt(0.5))
KAPPA = 2.081450                  # HW-calibrated mean of the avg2 ripple
LNK = float(np.log(KAPPA))
DVE_JS = frozenset((1, 3, 5, 7, 9, 11, 13, 15))  # s-chunks on the DVE exp path

_TCNT = [0]


def T(pool, shape, tag, bufs=None, dt=F32):
    _TCNT[0] += 1
    kw = dict(tag=tag, name=f"{tag}_{_TCNT[0]}")
    if bufs is not None:
        kw["bufs"] = bufs
    return pool.tile(shape, dt, **kw)


def build_program(loop_iters=None):
    nc = bacc.Bacc("TRN2", target_bir_lowering=False, debug=False)

    xT = nc.dram_tensor("xT", [C, N], BF16, kind="ExternalInput").ap()
    yT = nc.dram_tensor("yT", [C, M], BF16, kind="ExternalInput").ap()
    wqkvT = nc.dram_tensor("wqkvT", [C, 2 * HPC * HD], BF16, kind="ExternalInput").ap()
    bqkv = nc.dram_tensor("bqkv", [128, 4], F32, kind="ExternalInput").ap()
    wkvT = nc.dram_tensor("wkvT", [C, HPC * HD], BF16, kind="ExternalInput").ap()
    wvxT = nc.dram_tensor("wvxT", [C, HPC * HD], BF16, kind="ExternalInput").ap()
    wvyT = nc.dram_tensor("wvyT", [C, HPC * HD], BF16, kind="ExternalInput").ap()
    bvx = nc.dram_tensor("bvx", [1, HPC * HD], BF16, kind="ExternalInput").ap()
    bvy = nc.dram_tensor("bvy", [1, HPC * HD], BF16, kind="ExternalInput").ap()
    ones1r = nc.dram_tensor("ones1r", [1, 128], BF16, kind="ExternalInput").ap()
    bkv = nc.dram_tensor("bkv", [128, 2], F32, kind="ExternalInput").ap()
    wprojT = nc.dram_tensor("wprojT", [128, 2, C], BF16, kind="ExternalInput").ap()
    onesb = nc.dram_tensor("onesb", [128, 2], R32, kind="ExternalInput").ap()
    wqn_b = nc.dram_tensor("wqn_b", [2, 128], R32, kind="ExternalInput").ap()
    wkn_b = nc.dram_tensor("wkn_b", [2, 128], R32, kind="ExternalInput").ap()
    ones1c = nc.dram_tensor("ones1c", [128, 1], BF16, kind="ExternalInput").ap()
    sel2 = nc.dram_tensor("sel2", [128, 2 * 128], R32, kind="ExternalInput").ap()
    outT = nc.dram_tensor("outT", [C, N], F32, kind="ExternalOutput").ap()

    with tile.TileContext(nc) as tc:
        with (
            tc.tile_pool(name="const", bufs=1) as const,
            tc.tile_pool(name="sing", bufs=1) as sing,
        ):
            yT_sb = T(const, [128, 8, M], "yT", dt=BF16)
            nc.sync.dma_start(yT_sb, yT.rearrange("(o p) f -> p o f", p=128))
            wkv_sb = T(const, [128, 8, HPC * HD], "wkv", dt=BF16)
            nc.sync.dma_start(wkv_sb, wkvT.rearrange("(o p) f -> p o f", p=128))
            wvy_sb = T(const, [128, 8, HPC * HD], "wvy", dt=BF16)
            nc.sync.dma_start(wvy_sb, wvyT.rearrange("(o p) f -> p o f", p=128))
            wvx_sb = T(const, [128, 8, HPC * HD], "wvx", dt=BF16)
            nc.sync.dma_start(wvx_sb, wvxT.rearrange("(o p) f -> p o f", p=128))
            bvx_sb = T(const, [1, HPC * HD], "bvx", dt=BF16)
            nc.sync.dma_start(bvx_sb, bvx)
            bvy_sb = T(const, [1, HPC * HD], "bvy", dt=BF16)
            nc.sync.dma_start(bvy_sb, bvy)
            ones1_sb = T(const, [1, 128], "ones1r", dt=BF16)
            nc.sync.dma_start(ones1_sb, ones1r)
            wqkv_sb = T(const, [128, 8, 2 * HPC * HD], "wqkv", dt=BF16)
            wqkv_r = wqkvT.rearrange("(o p) f -> p o f", p=128)
            for cc in range(8):
                nc.sync.dma_start(wqkv_sb[:, cc], wqkv_r[:, cc])
            wproj_sb = T(const, [128, 2, C], "wproj", dt=BF16)
            nc.sync.dma_start(wproj_sb, wprojT)
            bqkv_sb = T(const, [128, 4], "bqkv")
            nc.sync.dma_start(bqkv_sb, bqkv)
            bkv_sb = T(const, [128, 2], "bkv")
            nc.sync.dma_start(bkv_sb, bkv)
            onesb_sb = T(const, [128, 2], "onesb", dt=R32)
            nc.sync.dma_start(onesb_sb, onesb)
            wqn_sb = T(const, [2, 128], "wqn_b", dt=R32)
            nc.sync.dma_start(wqn_sb, wqn_b)
            wkn_sb = T(const, [2, 128], "wkn_b", dt=R32)
            nc.sync.dma_start(wkn_sb, wkn_b)
            ones1c_sb = T(const, [128, 1], "ones1c", dt=BF16)
            nc.sync.dma_start(ones1c_sb, ones1c)
            sel_sb = T(const, [128, 2, 128], "sel2", dt=R32)
            nc.sync.dma_start(sel_sb, sel2.rearrange("p (a b) -> p a b", b=128))
            eps_sb = T(const, [128, 1], "epsc")
            nc.vector.memset(eps_sb, float(EPS))
            zero_sb = T(const, [128, 1], "zeroc")
            nc.vector.memset(zero_sb, 0.0)
            lnk_sb = T(const, [128, 1], "lnkc")
            nc.vector.memset(lnk_sb, LNK)

            # persistent activations
            qT = T(sing, [128, 2, N], "qT", dt=BF16)     # [idx*64+d, hp, n]
            kT = T(sing, [128, 2, S], "kT", dt=BF16)
            vS = T(sing, [128, NCH, HPC * HD], "vS", dt=BF16)  # [s%128, s//128, h*64+d]

            def norm_chunk(pool_ps, pool_wk, psum, bias_ap, w_lhsT, dest):
                """dest = (psum + bias) * rsqrt(mean_d((psum+bias)^2)+eps) * w"""
                nsz = psum.shape[-1]
                tb = T(pool_wk, [128, NT], "w")[:, :nsz]
                nc.vector.tensor_scalar_add(tb, psum, bias_ap)
                sq = T(pool_wk, [128, NT], "w", dt=R32)[:, :nsz]
                nc.vector.tensor_mul(sq, tb, tb)
                ssp = T(pool_ps, [2, NT], "paux", bufs=3)[:, :nsz]
                nc.tensor.matmul(ssp, onesb_sb, sq, start=True, stop=True)
                lnv = T(pool_wk, [2, NT], "w2", bufs=8)[:, :nsz]
                nc.scalar.activation(
                    lnv, ssp, AF.Ln, bias=eps_sb[0:2], scale=1.0 / HD
                )
                rmsv = T(pool_wk, [2, NT], "w2", bufs=8, dt=R32)[:, :nsz]
                nc.scalar.activation(rmsv, lnv, AF.Exp, bias=zero_sb[0:2], scale=-0.5)
                rbc = T(pool_ps, [128, NT], "paux", bufs=3)[:, :nsz]
                nc.tensor.matmul(rbc, w_lhsT, rmsv, start=True, stop=True)
                nc.vector.tensor_mul(dest, tb, rbc)

            def v_proj(pool_ps, src_sb, t, w_sb, b_sb, j):
                """vS[:, j] = (src.T @ wv + bv) directly in [s, d] layout."""
                pv = T(pool_ps, [128, HPC * HD], "pmain", bufs=4)
                for cc in range(8):
                    nc.tensor.matmul(
                        pv,
                        src_sb[:, cc, t * 128 : (t + 1) * 128],
                        w_sb[:, cc, :],
                        start=(cc == 0),
                        stop=False,
                    )
                nc.tensor.matmul(pv, ones1_sb, b_sb, start=False, stop=True)
                nc.vector.tensor_copy(out=vS[:, j, :], in_=pv)

            # ---- phase 1: KV projection of y (text tokens -> kv rows 2048..2303)
            import contextlib
            with contextlib.ExitStack() as _les:
                if loop_iters is not None:
                    _les.enter_context(tc.For_i(0, loop_iters, 1))
                with (
                    tc.tile_pool(name="pp12", bufs=3, space="PSUM") as pp12,
                    tc.tile_pool(name="wk", bufs=12) as wk,
                ):
                    for mc in range(2):  # [k01, k23]
                        ps = T(pp12, [128, NT], "pmain", bufs=4)[:, :M]
                        for cc in range(8):
                            nc.tensor.matmul(
                                ps,
                                wkv_sb[:, cc, mc * 128 : (mc + 1) * 128],
                                yT_sb[:, cc, :],
                                start=(cc == 0),
                                stop=(cc == 7),
                            )
                        norm_chunk(
                            pp12, wk, ps, bkv_sb[:, mc : mc + 1],
                            wkn_sb, kT[:, mc, N : N + M],
                        )
                    for t in range(2):
                        v_proj(pp12, yT_sb, t, wvy_sb, bvy_sb, 16 + t)

                    # ---- phase 2: QKV projection of x
                    with tc.tile_pool(name="xin", bufs=2) as xin:
                        for nt in range(N // NT):
                            nsl = slice(nt * NT, (nt + 1) * NT)
                            xc = T(xin, [128, 8, NT], "xc", dt=BF16)
                            nc.sync.dma_start(
                                xc, xT.rearrange("(o p) f -> p o f", p=128)[:, :, nsl]
                            )
                            for mc in range(4):  # [q01,q23,k01,k23]
                                ps = T(pp12, [128, NT], "pmain", bufs=4)
                                for cc in range(8):
                                    nc.tensor.matmul(
                                        ps,
                                        wqkv_sb[:, cc, mc * 128 : (mc + 1) * 128],
                                        xc[:, cc, :],
                                        start=(cc == 0),
                                        stop=(cc == 7),
                                    )
                                bias_ap = bqkv_sb[:, mc : mc + 1]
                                if mc < 2:
                                    norm_chunk(pp12, wk, ps, bias_ap,
                                               wqn_sb, qT[:, mc, nsl])
                                else:
                                    norm_chunk(pp12, wk, ps, bias_ap,
                                               wkn_sb, kT[:, mc - 2, nsl])
                            for t in range(4):
                                v_proj(pp12, xc, t, wvx_sb, bvx_sb, nt * 4 + t)

                # ---- phase 3+4: attention + output projection, per query tile
                with (
                    tc.tile_pool(name="pa_pl", bufs=4, space="PSUM") as pa_pl,
                    tc.tile_pool(name="pa_av", bufs=2, space="PSUM") as pa_av,
                    tc.tile_pool(name="pa_aux", bufs=2, space="PSUM") as pa_aux,
                    tc.tile_pool(name="atp", bufs=6) as atp,
                    tc.tile_pool(name="outp", bufs=2) as outp,
                    tc.tile_pool(name="osp", bufs=3) as osp,
                ):
                    for nt in range(N // NT):
                        nsl = slice(nt * NT, (nt + 1) * NT)
                        ot = T(outp, [128, 2, NT], "ot", dt=BF16)
                        av_ps = [T(pa_av, [128, NT], "av", bufs=2) for _ in range(2)]
                        den_ps = T(pa_aux, [128, NT], "aux", bufs=2)
                        nc.vector.memset(den_ps, 0.0)
                        for j in range(NCH):
                            at_t = [[None, None], [None, None]]
                            for hp in range(2):
                                for idx in range(2):
                                    prt = slice(64 * idx, 64 * idx + 64)
                                    pl = T(pa_pl, [128, NT], "pl", bufs=4)
                                    nc.tensor.matmul(
                                        pl,
                                        kT[prt, hp, j * 128 : (j + 1) * 128],
                                        qT[prt, hp, nsl],
                                        start=True, stop=True,
                                        tile_position=(64 * idx, 0),
                                    )
                                    at = T(atp, [128, NT], "at", dt=BF16)
                                    if j in DVE_JS:
                                        y0 = T(atp, [128, NT], "ys", bufs=4,
                                               dt=I16)
                                        nc.vector.tensor_scalar(
                                            y0, pl, float(ALPHA), float(BETA),
                                            op0=MUL, op1=ADD,
                                        )
                                        y1 = T(atp, [128, NT], "ys", bufs=4,
                                               dt=I16)
                                        nc.vector.tensor_scalar(
                                            y1, pl, float(ALPHA),
                                            float(BETA + 64.0),
                                            op0=MUL, op1=ADD,
                                        )
                                        nc.vector.scalar_tensor_tensor(
                                            at, y1.bitcast(BF16), SQ05,
                                            y0.bitcast(BF16),
                                            op0=MUL, op1=ADD,
                                        )
                                    else:
                                        nc.scalar.activation(
                                            at, pl, AF.Exp, bias=lnk_sb[:],
                                            scale=SCALE,
                                        )
                                    at_t[hp][idx] = at
                            for hp in range(2):
                                for idx in range(2):
                                    h = 2 * hp + idx
                                    nc.tensor.matmul(
                                        av_ps[hp][64 * idx : 64 * idx + 64, :],
                                        vS[:, j, 64 * h : 64 * h + 64],
                                        at_t[hp][idx],
                                        start=(j == 0), stop=(j == NCH - 1),
                                        tile_position=(0, 64 * idx),
                                    )
                            for h in range(4):
                                nc.tensor.matmul(
                                    den_ps[32 * h : 32 * h + 1, :],
                                    ones1c_sb,
                                    at_t[h // 2][h % 2],
                                    start=False,
                                    stop=(j == NCH - 1 and h == 3),
                                    tile_position=(0, 32 * h),
                                )
                        den_sb = T(osp, [128, NT], "den", dt=R32)
                        nc.scalar.copy(den_sb, den_ps)
                        for hp in range(2):
                            dbc = T(pa_pl, [128, NT], "pl", bufs=4)
                            nc.tensor.matmul(
                                dbc, sel_sb[:, hp, :], den_sb,
                                start=True, stop=True,
                            )
                            rbc = T(osp, [128, NT], "rbc")
                            nc.vector.reciprocal_approx_fast(rbc, dbc)
                            nc.vector.tensor_mul(ot[:, hp, :], av_ps[hp], rbc)

                        for oc in range(8):
                            po = T(pa_pl, [128, NT], "pl", bufs=4)
                            for cc in range(2):
                                nc.tensor.matmul(
                                    po,
                                    wproj_sb[:, cc, oc * 128 : (oc + 1) * 128],
                                    ot[:, cc, :],
                                    start=(cc == 0), stop=(cc == 1),
                                )
                            ob = T(osp, [128, NT], "ob")
                            nc.vector.tensor_copy(ob, po)
                            nc.sync.dma_start(
                                outT.rearrange("(o p) f -> p o f", p=128)[:, oc, nsl],
                                ob,
                            )
    _orig = bacc.get_activation_tables

    def _tables(arch):
        t = _orig(arch)
        return {
            name: (set() if name in ("exp_and_others", "natural_log",
                                     "exp_and_friends") else fns)
            for name, fns in t.items()
        }

    bacc.get_activation_tables = _tables
    try:
        nc.compile()
    finally:
        bacc.get_activation_tables = _orig
    return nc


_PROGRAM = None


def _get_program():
    global _PROGRAM
    if _PROGRAM is None:
        _PROGRAM = build_program()
    return _PROGRAM


def _make_in_maps(x, y, qkv_w, qkv_b, kv_w, kv_b, qn_w, kn_w, proj_w, proj_b):
    f = np.float32
    bf = ml_dtypes.bfloat16
    onesb = np.zeros((128, 2), f)
    onesb[0:64, 0] = 1.0
    onesb[64:128, 1] = 1.0
    wqn = np.zeros((2, 128), f)
    wqn[0, 0:64] = qn_w
    wqn[1, 64:128] = qn_w
    wkn = np.zeros((2, 128), f)
    wkn[0, 0:64] = kn_w
    wkn[1, 64:128] = kn_w
    sel2 = np.zeros((128, 2, 128), f)
    for hp in range(2):
        for a in range(2):
            sel2[32 * (2 * hp + a), hp, 64 * a : 64 * a + 64] = 1.0

    in_maps = []
    for core in range(8):
        b, g = divmod(core, 4)
        qs = slice(g * 256, (g + 1) * 256)
        wqkv = np.concatenate([qkv_w[qs], qkv_w[1024:2048][qs]], axis=0)
        bq = np.concatenate([qkv_b[qs], qkv_b[1024:2048][qs]])
        wkv = kv_w[qs]
        bk = kv_b[qs]
        wvx = qkv_w[2048:3072][qs]
        bvxv = qkv_b[2048:3072][qs]
        wvy = kv_w[1024:2048][qs]
        bvyv = kv_b[1024:2048][qs]
        wproj = np.ascontiguousarray(proj_w[:, qs].T, f)  # [256, 1024]
        in_maps.append(
            {
                "xT": np.ascontiguousarray(x[b].T).astype(bf),
                "yT": np.ascontiguousarray(y[b].T).astype(bf),
                "wqkvT": np.ascontiguousarray(wqkv.T).astype(bf),
                "bqkv": np.ascontiguousarray(bq.reshape(4, 128).T, f),
                "wkvT": np.ascontiguousarray(wkv.T).astype(bf),
                "bkv": np.ascontiguousarray(bk.reshape(2, 128).T, f),
                "wvxT": np.ascontiguousarray(wvx.T).astype(bf),
                "bvx": bvxv.reshape(1, 256).astype(bf),
                "wvyT": np.ascontiguousarray(wvy.T).astype(bf),
                "bvy": bvyv.reshape(1, 256).astype(bf),
                "ones1r": np.ones((1, 128), bf),
                "wprojT": np.ascontiguousarray(wproj.reshape(2, 128, C).transpose(1, 0, 2)).astype(bf),
                "onesb": onesb,
                "wqn_b": wqn,
                "wkn_b": wkn,
                "ones1c": np.ones((128, 1), bf),
                "sel2": np.ascontiguousarray(sel2.reshape(128, 256)),
            }
        )
    return in_maps


def run_cores(inputs, trace=False, **kwargs):
    nc = _get_program()
    in_maps = _make_in_maps(**{k: np.asarray(v, np.float32) for k, v in inputs.items()})
    return run_bass_kernel_spmd(
        nc, in_maps, core_ids=list(range(8)), trace=trace, **kwargs
    )


def kernel(**inputs):
    proj_b = np.asarray(inputs["proj_b"], np.float32)
    res = run_cores(inputs).results
    out = np.zeros((B, N, C), np.float32)
    for core in range(8):
        b = core // 4
        out[b] += res[core]["outT"].T
    out += proj_b[None, None, :]
    return out


# revision 8
# speedup vs baseline: 3.4638x; 1.0967x over previous
"""Trainium2 Bass kernel for nn_CrossAttention (B=2, N=2048, M=256, C=1024, H=16).

Sharding: 8 cores = 2 batches x 4 head-groups (4 heads each). Each core
computes its heads' QKV/KV projections, qk-RMSNorm, attention and a partial
output projection over its 256 channels; the host sums the 4 partials per
batch (the all-reduce) and adds proj_b.

v2 design vs baseline:
- all big matmuls in bf16 (same PE rate as fp32r, half the SBUF/DMA)
- softmax exp on the Vector engine via Schraudolph bit-trick
  (bf16 = bitcast(int16(alpha*x + beta))) instead of ScalarE activation:
  ~3x faster per tile and frees the Scalar engine
- AV matmuls pack 2 heads via column tiling (64+64 of 128 cols)
- softmax denominators via 4-way col-tiled ones-matmuls into a shared bank
- output projection packs 2 head-pairs on the contraction dim (128 rows)
- PSUM->SBUF evacuations moved to the otherwise-idle Scalar engine
"""

import sys

sys.path.insert(0, "/opt/trn_rl_repo")

import numpy as np  # noqa: E402
import ml_dtypes  # noqa: E402

import concourse.bass as bass  # noqa: E402
import concourse.tile as tile  # noqa: E402
from concourse import bacc, mybir  # noqa: E402
from concourse.bass_utils import run_bass_kernel_spmd  # noqa: E402

F32 = mybir.dt.float32
R32 = mybir.dt.float32r
BF16 = mybir.dt.bfloat16
I16 = mybir.dt.int16
AF = mybir.ActivationFunctionType
MUL = mybir.AluOpType.mult
ADD = mybir.AluOpType.add

H = 16
B = 2
N = 2048          # image tokens
M = 256           # text tokens
C = 1024
HD = 64           # head dim
EPS = 1e-6
S = N + M         # 2304 kv length
NCH = S // 128    # 18 kv chunks
HPC = 4           # heads per core
NT = 512          # query tile
SCALE = HD ** -0.5
LN2 = float(np.log(2.0))
ALPHA = SCALE * 128.0 / LN2       # fold logit scale into the exp bit-trick
BETA = 127.0 * 128.0              # bf16 exponent bias in int16 units
SQ05 = float(np.sqrt(0.5))
KAPPA = 2.081450                  # HW-calibrated mean of the avg2 ripple
LNK = float(np.log(KAPPA))
NJP = NCH // 2                    # 9 pairs of s-chunks
# exp-engine assignment per granule (jp, hp), 18 granules per query tile:
# ~10/18 on ScalarE (exact exp), ~8/18 on VectorE (avg2 Schraudolph)
DVE_JP = frozenset((1, 3, 5, 7))  # jp indices whose exp runs on VectorE

_TCNT = [0]


def T(pool, shape, tag, bufs=None, dt=F32):
    _TCNT[0] += 1
    kw = dict(tag=tag, name=f"{tag}_{_TCNT[0]}")
    if bufs is not None:
        kw["bufs"] = bufs
    return pool.tile(shape, dt, **kw)


def build_program(loop_iters=None, only=None):
    nc = bacc.Bacc("TRN2", target_bir_lowering=False, debug=False)

    xT = nc.dram_tensor("xT", [C, N], BF16, kind="ExternalInput").ap()
    yT = nc.dram_tensor("yT", [C, M], BF16, kind="ExternalInput").ap()
    wqkvT = nc.dram_tensor("wqkvT", [C, 2 * HPC * HD], BF16, kind="ExternalInput").ap()
    bqkv = nc.dram_tensor("bqkv", [128, 4], F32, kind="ExternalInput").ap()
    wkvT = nc.dram_tensor("wkvT", [C, HPC * HD], BF16, kind="ExternalInput").ap()
    wvxT = nc.dram_tensor("wvxT", [C, HPC * HD], BF16, kind="ExternalInput").ap()
    wvyT = nc.dram_tensor("wvyT", [C, HPC * HD], BF16, kind="ExternalInput").ap()
    bvx = nc.dram_tensor("bvx", [1, HPC * HD], BF16, kind="ExternalInput").ap()
    bvy = nc.dram_tensor("bvy", [1, HPC * HD], BF16, kind="ExternalInput").ap()
    ones1r = nc.dram_tensor("ones1r", [1, 128], BF16, kind="ExternalInput").ap()
    bkv = nc.dram_tensor("bkv", [128, 2], F32, kind="ExternalInput").ap()
    wprojT = nc.dram_tensor("wprojT", [128, 2, C], BF16, kind="ExternalInput").ap()
    onesb = nc.dram_tensor("onesb", [128, 2], R32, kind="ExternalInput").ap()
    wqn_b = nc.dram_tensor("wqn_b", [2, 128], R32, kind="ExternalInput").ap()
    wkn_b = nc.dram_tensor("wkn_b", [2, 128], R32, kind="ExternalInput").ap()
    ones1c = nc.dram_tensor("ones1c", [128, 1], BF16, kind="ExternalInput").ap()
    sel2 = nc.dram_tensor("sel2", [128, 2 * 128], R32, kind="ExternalInput").ap()
    outT = nc.dram_tensor("outT", [C, N], BF16, kind="ExternalOutput").ap()

    with tile.TileContext(nc) as tc:
        with (
            tc.tile_pool(name="const", bufs=1) as const,
            tc.tile_pool(name="sing", bufs=1) as sing,
        ):
            yT_sb = T(const, [128, 8, M], "yT", dt=BF16)
            nc.sync.dma_start(yT_sb, yT.rearrange("(o p) f -> p o f", p=128))
            wkv_sb = T(const, [128, 8, HPC * HD], "wkv", dt=BF16)
            nc.sync.dma_start(wkv_sb, wkvT.rearrange("(o p) f -> p o f", p=128))
            wvy_sb = T(const, [128, 8, HPC * HD], "wvy", dt=BF16)
            nc.sync.dma_start(wvy_sb, wvyT.rearrange("(o p) f -> p o f", p=128))
            wvx_sb = T(const, [128, 8, HPC * HD], "wvx", dt=BF16)
            nc.sync.dma_start(wvx_sb, wvxT.rearrange("(o p) f -> p o f", p=128))
            bvx_sb = T(const, [1, HPC * HD], "bvx", dt=BF16)
            nc.sync.dma_start(bvx_sb, bvx)
            bvy_sb = T(const, [1, HPC * HD], "bvy", dt=BF16)
            nc.sync.dma_start(bvy_sb, bvy)
            ones1_sb = T(const, [1, 128], "ones1r", dt=BF16)
            nc.sync.dma_start(ones1_sb, ones1r)
            wqkv_sb = T(const, [128, 8, 2 * HPC * HD], "wqkv", dt=BF16)
            wqkv_r = wqkvT.rearrange("(o p) f -> p o f", p=128)
            for cc in range(8):
                nc.sync.dma_start(wqkv_sb[:, cc], wqkv_r[:, cc])
            wproj_sb = T(const, [128, 2, C], "wproj", dt=BF16)
            nc.sync.dma_start(wproj_sb, wprojT)
            bqkv_sb = T(const, [128, 4], "bqkv")
            nc.sync.dma_start(bqkv_sb, bqkv)
            bkv_sb = T(const, [128, 2], "bkv")
            nc.sync.dma_start(bkv_sb, bkv)
            onesb_sb = T(const, [128, 2], "onesb", dt=R32)
            nc.sync.dma_start(onesb_sb, onesb)
            wqn_sb = T(const, [2, 128], "wqn_b", dt=R32)
            nc.sync.dma_start(wqn_sb, wqn_b)
            wkn_sb = T(const, [2, 128], "wkn_b", dt=R32)
            nc.sync.dma_start(wkn_sb, wkn_b)
            ones1c_sb = T(const, [128, 1], "ones1c", dt=BF16)
            nc.sync.dma_start(ones1c_sb, ones1c)
            sel_sb = T(const, [128, 2, 128], "sel2", dt=R32)
            nc.sync.dma_start(sel_sb, sel2.rearrange("p (a b) -> p a b", b=128))
            eps_sb = T(const, [128, 1], "epsc")
            nc.vector.memset(eps_sb, float(EPS))
            zero_sb = T(const, [128, 1], "zeroc")
            nc.vector.memset(zero_sb, 0.0)
            lnk_sb = T(const, [128, 1], "lnkc")
            nc.vector.memset(lnk_sb, LNK)

            # persistent activations
            qT = T(sing, [128, 2, N], "qT", dt=BF16)     # [idx*64+d, hp, n]
            kT = T(sing, [128, 2, S], "kT", dt=BF16)
            vS = T(sing, [128, NCH, HPC * HD], "vS", dt=BF16)  # [s%128, s//128, h*64+d]

            def norm_chunk(pool_ps, pool_wk, psum, bias_ap, w_lhsT, dest):
                """dest = (psum + bias) * rsqrt(mean_d((psum+bias)^2)+eps) * w"""
                nsz = psum.shape[-1]
                tb = T(pool_wk, [128, NT], "w")[:, :nsz]
                nc.vector.tensor_scalar_add(tb, psum, bias_ap)
                sq = T(pool_wk, [128, NT], "w", dt=R32)[:, :nsz]
                nc.vector.tensor_mul(sq, tb, tb)
                ssp = T(pool_ps, [2, NT], "paux", bufs=3)[:, :nsz]
                nc.tensor.matmul(ssp, onesb_sb, sq, start=True, stop=True)
                lnv = T(pool_wk, [2, NT], "w2", bufs=8)[:, :nsz]
                nc.scalar.activation(
                    lnv, ssp, AF.Ln, bias=eps_sb[0:2], scale=1.0 / HD
                )
                rmsv = T(pool_wk, [2, NT], "w2", bufs=8, dt=R32)[:, :nsz]
                nc.scalar.activation(rmsv, lnv, AF.Exp, bias=zero_sb[0:2], scale=-0.5)
                rbc = T(pool_ps, [128, NT], "paux", bufs=3)[:, :nsz]
                nc.tensor.matmul(rbc, w_lhsT, rmsv, start=True, stop=True)
                nc.vector.tensor_mul(dest, tb, rbc)

            def v_proj(pool_ps, src_sb, t, w_sb, b_sb, j):
                """vS[:, j] = (src.T @ wv + bv) directly in [s, d] layout."""
                pv = T(pool_ps, [128, HPC * HD], "pmain", bufs=4)
                for cc in range(8):
                    nc.tensor.matmul(
                        pv,
                        src_sb[:, cc, t * 128 : (t + 1) * 128],
                        w_sb[:, cc, :],
                        start=(cc == 0),
                        stop=False,
                    )
                nc.tensor.matmul(pv, ones1_sb, b_sb, start=False, stop=True)
                nc.vector.tensor_copy(out=vS[:, j, :], in_=pv)

            # ---- phase 1: KV projection of y (text tokens -> kv rows 2048..2303)
            import contextlib
            with contextlib.ExitStack() as _les:
                if loop_iters is not None and only is None:
                    _les.enter_context(tc.For_i(0, loop_iters, 1))
                elif loop_iters is not None and only == "p12":
                    _les.enter_context(tc.For_i(0, loop_iters, 1))
                with (
                    tc.tile_pool(name="pp12", bufs=3, space="PSUM") as pp12,
                    tc.tile_pool(name="wk", bufs=12) as wk,
                ):
                    for mc in range(2):  # [k01, k23]
                        ps = T(pp12, [128, NT], "pmain", bufs=4)[:, :M]
                        for cc in range(8):
                            nc.tensor.matmul(
                                ps,
                                wkv_sb[:, cc, mc * 128 : (mc + 1) * 128],
                                yT_sb[:, cc, :],
                                start=(cc == 0),
                                stop=(cc == 7),
                            )
                        norm_chunk(
                            pp12, wk, ps, bkv_sb[:, mc : mc + 1],
                            wkn_sb, kT[:, mc, N : N + M],
                        )
                    for t in range(2):
                        v_proj(pp12, yT_sb, t, wvy_sb, bvy_sb, 16 + t)

                    # ---- phase 2: QKV projection of x
                    with tc.tile_pool(name="xin", bufs=2) as xin:
                        for nt in range(N // NT):
                            nsl = slice(nt * NT, (nt + 1) * NT)
                            xc = T(xin, [128, 8, NT], "xc", dt=BF16)
                            nc.sync.dma_start(
                                xc, xT.rearrange("(o p) f -> p o f", p=128)[:, :, nsl]
                            )
                            for mc in range(4):  # [q01,q23,k01,k23]
                                ps = T(pp12, [128, NT], "pmain", bufs=4)
                                for cc in range(8):
                                    nc.tensor.matmul(
                                        ps,
                                        wqkv_sb[:, cc, mc * 128 : (mc + 1) * 128],
                                        xc[:, cc, :],
                                        start=(cc == 0),
                                        stop=(cc == 7),
                                    )
                                bias_ap = bqkv_sb[:, mc : mc + 1]
                                if mc < 2:
                                    norm_chunk(pp12, wk, ps, bias_ap,
                                               wqn_sb, qT[:, mc, nsl])
                                else:
                                    norm_chunk(pp12, wk, ps, bias_ap,
                                               wkn_sb, kT[:, mc - 2, nsl])
                            for t in range(4):
                                v_proj(pp12, xc, t, wvx_sb, bvx_sb, nt * 4 + t)

                if only == "p12":
                    return_phase3 = False
                else:
                    return_phase3 = True
                # ---- phase 3+4: attention + output projection, per query tile
                # Pipeline granule = (jp, hp): two [128, 2*NT] bf16 logits
                # tiles (idx 0/1), exp'd on ScalarE or VectorE, then AV +
                # denominator matmuls.  Granules alternate between two PSUM
                # pools so the in-order PE queue never head-of-line blocks on
                # the exp engine of the previous granule.
                with (
                    tc.tile_pool(name="pa_plA", bufs=2, space="PSUM") as pa_plA,
                    tc.tile_pool(name="pa_plB", bufs=2, space="PSUM") as pa_plB,
                    tc.tile_pool(name="pa_av", bufs=2, space="PSUM") as pa_av,
                    tc.tile_pool(name="pa_aux", bufs=2, space="PSUM") as pa_aux,
                    tc.tile_pool(name="atp", bufs=12) as atp,
                    tc.tile_pool(name="outp", bufs=2) as outp,
                    tc.tile_pool(name="osp", bufs=3) as osp,
                ):
                    pl_pools = [pa_plA, pa_plB]
                    for nt in range(N // NT):
                        nsl = slice(nt * NT, (nt + 1) * NT)
                        ot = T(outp, [128, 2, NT], "ot", dt=BF16)
                        av_ps = [T(pa_av, [128, NT], "av", bufs=2) for _ in range(2)]
                        den_ps = T(pa_aux, [128, NT], "aux", bufs=2)
                        nc.vector.memset(den_ps, 0.0)
                        for jp in range(NJP):
                            at_jp = [[None, None], [None, None]]
                            for hp in range(2):
                                pls = []
                                for idx in range(2):
                                    pls.append(
                                        T(pl_pools[idx], [128, 2 * NT], "pl",
                                          bufs=1, dt=F32)
                                    )
                                for u in range(2):
                                    for idx in range(2):
                                        prt = slice(64 * idx, 64 * idx + 64)
                                        j = 2 * jp + u
                                        nc.tensor.matmul(
                                            pls[idx][:, u * NT : (u + 1) * NT],
                                            kT[prt, hp, j * 128 : (j + 1) * 128],
                                            qT[prt, hp, nsl],
                                            start=True, stop=True,
                                            tile_position=(64 * idx, 0),
                                        )
                                for idx in range(2):
                                    at = T(atp, [128, 2 * NT], "at", dt=BF16)
                                    if jp in DVE_JP:
                                        y0 = T(atp, [128, 2 * NT], "ys",
                                               bufs=4, dt=I16)
                                        nc.vector.tensor_scalar(
                                            y0, pls[idx], float(ALPHA),
                                            float(BETA), op0=MUL, op1=ADD,
                                        )
                                        y1 = T(atp, [128, 2 * NT], "ys",
                                               bufs=4, dt=I16)
                                        nc.vector.tensor_scalar(
                                            y1, pls[idx], float(ALPHA),
                                            float(BETA + 64.0),
                                            op0=MUL, op1=ADD,
                                        )
                                        nc.vector.scalar_tensor_tensor(
                                            at, y1.bitcast(BF16), SQ05,
                                            y0.bitcast(BF16),
                                            op0=MUL, op1=ADD,
                                        )
                                    else:
                                        nc.scalar.activation(
                                            at, pls[idx], AF.Exp,
                                            bias=lnk_sb[:], scale=SCALE,
                                        )
                                    at_jp[hp][idx] = at
                                for u in range(2):
                                    j = 2 * jp + u
                                    usl = slice(u * NT, (u + 1) * NT)
                                    for idx in range(2):
                                        h = 2 * hp + idx
                                        nc.tensor.matmul(
                                            av_ps[hp][64 * idx : 64 * idx + 64, :],
                                            vS[:, j, 64 * h : 64 * h + 64],
                                            at_jp[hp][idx][:, usl],
                                            start=(j == 0),
                                            stop=(j == NCH - 1),
                                            tile_position=(0, 64 * idx),
                                        )
                            for u in range(2):
                                j = 2 * jp + u
                                usl = slice(u * NT, (u + 1) * NT)
                                for h in range(4):
                                    nc.tensor.matmul(
                                        den_ps[32 * h : 32 * h + 1, :],
                                        ones1c_sb,
                                        at_jp[h // 2][h % 2][:, usl],
                                        start=False,
                                        stop=(j == NCH - 1 and h == 3),
                                        tile_position=(0, 32 * h),
                                    )
                        den_sb = T(osp, [128, NT], "den", dt=R32)
                        nc.scalar.copy(den_sb, den_ps)
                        for hp in range(2):
                            dbc = T(pl_pools[hp], [128, NT], "pl", bufs=1, dt=F32)
                            nc.tensor.matmul(
                                dbc, sel_sb[:, hp, :], den_sb,
                                start=True, stop=True,
                            )
                            rbc = T(osp, [128, NT], "rbc")
                            nc.vector.reciprocal_approx_fast(rbc, dbc)
                            nc.vector.tensor_mul(ot[:, hp, :], av_ps[hp], rbc)

                        for oc in range(8):
                            po = T(pl_pools[oc % 2], [128, NT], "pl", bufs=1, dt=F32)
                            for cc in range(2):
                                nc.tensor.matmul(
                                    po,
                                    wproj_sb[:, cc, oc * 128 : (oc + 1) * 128],
                                    ot[:, cc, :],
                                    start=(cc == 0), stop=(cc == 1),
                                )
                            ob = T(osp, [128, NT], "ob", dt=BF16)
                            nc.vector.tensor_copy(ob, po)
                            nc.sync.dma_start(
                                outT.rearrange("(o p) f -> p o f", p=128)[:, oc, nsl],
                                ob,
                            )
    _orig = bacc.get_activation_tables

    def _tables(arch):
        t = _orig(arch)
        return {
            name: (set() if name in ("exp_and_others", "natural_log",
                                     "exp_and_friends") else fns)
            for name, fns in t.items()
        }

    bacc.get_activation_tables = _tables
    try:
        nc.compile()
    finally:
        bacc.get_activation_tables = _orig
    return nc


_PROGRAM = None


def _get_program():
    global _PROGRAM
    if _PROGRAM is None:
        _PROGRAM = build_program()
    return _PROGRAM


def _make_in_maps(x, y, qkv_w, qkv_b, kv_w, kv_b, qn_w, kn_w, proj_w, proj_b):
    f = np.float32
    bf = ml_dtypes.bfloat16
    onesb = np.zeros((128, 2), f)
    onesb[0:64, 0] = 1.0
    onesb[64:128, 1] = 1.0
    wqn = np.zeros((2, 128), f)
    wqn[0, 0:64] = qn_w
    wqn[1, 64:128] = qn_w
    wkn = np.zeros((2, 128), f)
    wkn[0, 0:64] = kn_w
    wkn[1, 64:128] = kn_w
    sel2 = np.zeros((128, 2, 128), f)
    for hp in range(2):
        for a in range(2):
            sel2[32 * (2 * hp + a), hp, 64 * a : 64 * a + 64] = 1.0

    in_maps = []
    for core in range(8):
        b, g = divmod(core, 4)
        qs = slice(g * 256, (g + 1) * 256)
        wqkv = np.concatenate([qkv_w[qs], qkv_w[1024:2048][qs]], axis=0)
        bq = np.concatenate([qkv_b[qs], qkv_b[1024:2048][qs]])
        wkv = kv_w[qs]
        bk = kv_b[qs]
        wvx = qkv_w[2048:3072][qs]
        bvxv = qkv_b[2048:3072][qs]
        wvy = kv_w[1024:2048][qs]
        bvyv = kv_b[1024:2048][qs]
        wproj = np.ascontiguousarray(proj_w[:, qs].T, f)  # [256, 1024]
        in_maps.append(
            {
                "xT": np.ascontiguousarray(x[b].T).astype(bf),
                "yT": np.ascontiguousarray(y[b].T).astype(bf),
                "wqkvT": np.ascontiguousarray(wqkv.T).astype(bf),
                "bqkv": np.ascontiguousarray(bq.reshape(4, 128).T, f),
                "wkvT": np.ascontiguousarray(wkv.T).astype(bf),
                "bkv": np.ascontiguousarray(bk.reshape(2, 128).T, f),
                "wvxT": np.ascontiguousarray(wvx.T).astype(bf),
                "bvx": bvxv.reshape(1, 256).astype(bf),
                "wvyT": np.ascontiguousarray(wvy.T).astype(bf),
                "bvy": bvyv.reshape(1, 256).astype(bf),
                "ones1r": np.ones((1, 128), bf),
                "wprojT": np.ascontiguousarray(wproj.reshape(2, 128, C).transpose(1, 0, 2)).astype(bf),
                "onesb": onesb,
                "wqn_b": wqn,
                "wkn_b": wkn,
                "ones1c": np.ones((128, 1), bf),
                "sel2": np.ascontiguousarray(sel2.reshape(128, 256)),
            }
        )
    return in_maps


def run_cores(inputs, trace=False, **kwargs):
    nc = _get_program()
    in_maps = _make_in_maps(**{k: np.asarray(v, np.float32) for k, v in inputs.items()})
    return run_bass_kernel_spmd(
        nc, in_maps, core_ids=list(range(8)), trace=trace, **kwargs
    )


def kernel(**inputs):
    proj_b = np.asarray(inputs["proj_b"], np.float32)
    res = run_cores(inputs).results
    out = np.zeros((B, N, C), np.float32)
    for core in range(8):
        b = core // 4
        out[b] += np.asarray(res[core]["outT"], np.float32).T
    out += proj_b[None, None, :]
    return out


# revision 13
# speedup vs baseline: 5.6003x; 1.6168x over previous
"""Trainium2 Bass kernel for nn_CrossAttention (B=2, N=2048, M=256, C=1024, H=16).

Sharding: 8 cores = 2 batches x 4 head-groups (4 heads each). Each core
computes its heads' QKV/KV projections, qk-RMSNorm, attention and a partial
output projection over its 256 channels; the host sums the 4 partials per
batch (the all-reduce) and adds proj_b.

Design notes:
- all big matmuls in bf16 (same PE rate as fp32r, half the SBUF/DMA)
- exp split: ~2/3 of softmax tiles exact on ScalarE, ~1/3 on VectorE via a
  one-instruction Schraudolph bit-trick (bf16 = bitcast(int16(a*x+b)));
  the ScalarE tiles carry a matching ln(kappa) bias so both paths share one
  global scale that cancels in the softmax ratio
- AV matmuls pack 2 heads via column tiling; denominators via 4-way
  col-tiled ones-matmuls into a shared bank (memset + start=False trick)
- output projection packs 2 head-pairs on the contraction dim
- QKV biases folded into the PE accumulation (bias-row x ones matmul)
- phase 3 emission is software-pipelined (logits of granule g, AV of g-1,
  denominators of g-2) so the in-order PE queue never head-of-line blocks
  on the exp engines; RMSNorm matmuls are staged the same way
"""

import sys

sys.path.insert(0, "/opt/trn_rl_repo")

import contextlib  # noqa: E402

import numpy as np  # noqa: E402
import ml_dtypes  # noqa: E402

import concourse.bass as bass  # noqa: E402
import concourse.tile as tile  # noqa: E402
from concourse import bacc, mybir  # noqa: E402
from concourse.bass_utils import run_bass_kernel_spmd  # noqa: E402

F32 = mybir.dt.float32
R32 = mybir.dt.float32r
BF16 = mybir.dt.bfloat16
I16 = mybir.dt.int16
AF = mybir.ActivationFunctionType
MUL = mybir.AluOpType.mult
ADD = mybir.AluOpType.add

H = 16
B = 2
N = 2048          # image tokens
M = 256           # text tokens
C = 1024
HD = 64           # head dim
EPS = 1e-6
S = N + M         # 2304 kv length
NCH = S // 128    # 18 kv chunks
HPC = 4           # heads per core
NT = 512          # query tile
SCALE = HD ** -0.5
LN2 = float(np.log(2.0))
ALPHA = SCALE * 128.0 / LN2       # fold logit scale into the exp bit-trick
BETA = 127.0 * 128.0              # bf16 exponent bias in int16 units
BETA_P = BETA - 7.0               # minimax-balanced plain-Schraudolph bias
KAPPA1 = 1.0020                   # mean ratio of plain Schraudolph vs exp
LNK = float(np.log(KAPPA1))
NJP = NCH // 2                    # 9 pairs of s-chunks

_TCNT = [0]


def T(pool, shape, tag, bufs=None, dt=F32):
    _TCNT[0] += 1
    kw = dict(tag=tag, name=f"{tag}_{_TCNT[0]}")
    if bufs is not None:
        kw["bufs"] = bufs
    return pool.tile(shape, dt, **kw)


def build_program(loop_iters=None, only=None, exp="hybrid"):
    nc = bacc.Bacc("TRN2", target_bir_lowering=False, debug=False)

    xT = nc.dram_tensor("xT", [C, N], BF16, kind="ExternalInput").ap()
    yT = nc.dram_tensor("yT", [C, M], BF16, kind="ExternalInput").ap()
    wqkvT = nc.dram_tensor("wqkvT", [C, 2 * HPC * HD], BF16, kind="ExternalInput").ap()
    bqr = nc.dram_tensor("bqr", [1, 512], BF16, kind="ExternalInput").ap()
    wkvT = nc.dram_tensor("wkvT", [C, HPC * HD], BF16, kind="ExternalInput").ap()
    bkr = nc.dram_tensor("bkr", [1, 256], BF16, kind="ExternalInput").ap()
    wvxT = nc.dram_tensor("wvxT", [C, HPC * HD], BF16, kind="ExternalInput").ap()
    wvyT = nc.dram_tensor("wvyT", [C, HPC * HD], BF16, kind="ExternalInput").ap()
    bvx = nc.dram_tensor("bvx", [1, HPC * HD], BF16, kind="ExternalInput").ap()
    bvy = nc.dram_tensor("bvy", [1, HPC * HD], BF16, kind="ExternalInput").ap()
    ones1r = nc.dram_tensor("ones1r", [1, 128], BF16, kind="ExternalInput").ap()
    wprojT = nc.dram_tensor("wprojT", [128, 2, C], BF16, kind="ExternalInput").ap()
    onesb = nc.dram_tensor("onesb", [128, 2], R32, kind="ExternalInput").ap()
    wqn_b = nc.dram_tensor("wqn_b", [2, 128], R32, kind="ExternalInput").ap()
    wkn_b = nc.dram_tensor("wkn_b", [2, 128], R32, kind="ExternalInput").ap()
    ones1c = nc.dram_tensor("ones1c", [128, 1], BF16, kind="ExternalInput").ap()
    sel2 = nc.dram_tensor("sel2", [128, 2 * 128], R32, kind="ExternalInput").ap()
    outT = nc.dram_tensor("outT", [C, N], BF16, kind="ExternalOutput").ap()

    with tile.TileContext(nc) as tc:
        with (
            tc.tile_pool(name="const", bufs=1) as const,
            tc.tile_pool(name="sing", bufs=1) as sing,
        ):
            yT_sb = T(const, [128, 8, M], "yT", dt=BF16)
            nc.sync.dma_start(yT_sb, yT.rearrange("(o p) f -> p o f", p=128))
            wkv_sb = T(const, [128, 8, HPC * HD], "wkv", dt=BF16)
            nc.sync.dma_start(wkv_sb, wkvT.rearrange("(o p) f -> p o f", p=128))
            wvy_sb = T(const, [128, 8, HPC * HD], "wvy", dt=BF16)
            nc.sync.dma_start(wvy_sb, wvyT.rearrange("(o p) f -> p o f", p=128))
            wvx_sb = T(const, [128, 8, HPC * HD], "wvx", dt=BF16)
            nc.sync.dma_start(wvx_sb, wvxT.rearrange("(o p) f -> p o f", p=128))
            bvx_sb = T(const, [1, HPC * HD], "bvx", dt=BF16)
            nc.sync.dma_start(bvx_sb, bvx)
            bvy_sb = T(const, [1, HPC * HD], "bvy", dt=BF16)
            nc.sync.dma_start(bvy_sb, bvy)
            ones1_sb = T(const, [1, 128], "ones1r", dt=BF16)
            nc.sync.dma_start(ones1_sb, ones1r)
            wqkv_sb = T(const, [128, 8, 2 * HPC * HD], "wqkv", dt=BF16)
            wqkv_r = wqkvT.rearrange("(o p) f -> p o f", p=128)
            for cc in range(8):
                nc.sync.dma_start(wqkv_sb[:, cc], wqkv_r[:, cc])
            wproj_sb = T(const, [128, 2, C], "wproj", dt=BF16)
            nc.sync.dma_start(wproj_sb, wprojT)
            bqr_sb = T(const, [1, 512], "bqr", dt=BF16)
            nc.sync.dma_start(bqr_sb, bqr)
            bkr_sb = T(const, [1, 256], "bkr", dt=BF16)
            nc.sync.dma_start(bkr_sb, bkr)
            onesNT_sb = T(const, [1, NT], "onesNT", dt=BF16)
            nc.vector.memset(onesNT_sb, 1.0)
            onesb_sb = T(const, [128, 2], "onesb", dt=R32)
            nc.sync.dma_start(onesb_sb, onesb)
            wqn_sb = T(const, [2, 128], "wqn_b", dt=R32)
            nc.sync.dma_start(wqn_sb, wqn_b)
            wkn_sb = T(const, [2, 128], "wkn_b", dt=R32)
            nc.sync.dma_start(wkn_sb, wkn_b)
            ones1c_sb = T(const, [128, 1], "ones1c", dt=BF16)
            nc.sync.dma_start(ones1c_sb, ones1c)
            sel_sb = T(const, [128, 2, 128], "sel2", dt=R32)
            nc.sync.dma_start(sel_sb, sel2.rearrange("p (a b) -> p a b", b=128))
            eps_sb = T(const, [128, 1], "epsc")
            nc.vector.memset(eps_sb, float(EPS))
            zero_sb = T(const, [128, 1], "zeroc")
            nc.vector.memset(zero_sb, 0.0)
            lnk_sb = T(const, [128, 1], "lnkc")
            nc.vector.memset(lnk_sb, LNK)

            # persistent activations
            qT = T(sing, [128, 2, N], "qT", dt=BF16)     # [idx*64+d, hp, n]
            kT = T(sing, [128, 2, S], "kT", dt=BF16)
            vS = T(sing, [128, NCH, HPC * HD], "vS", dt=BF16)  # [s%128, s//128, h*64+d]

            # RMSNorm as a 3-stage software pipeline so its PE matmuls
            # (ssp, rbc) never head-of-line block projection matmuls behind
            # a pending cross-engine dependency.
            _nq = {"items": [], "s1": 0, "s2": 0}

            def _norm_s1(pool_ps, pool_wk, it):
                nsz = it["nsz"]
                ssp = T(pool_ps, [2, NT], "paux", bufs=3)[:, :nsz]
                nc.tensor.matmul(ssp, onesb_sb, it["sq"], start=True, stop=True)
                lnv = T(pool_wk, [2, NT], "w2", bufs=8)[:, :nsz]
                nc.scalar.activation(
                    lnv, ssp, AF.Ln, bias=eps_sb[0:2], scale=1.0 / HD
                )
                rmsv = T(pool_wk, [2, NT], "w2", bufs=8, dt=R32)[:, :nsz]
                nc.scalar.activation(
                    rmsv, lnv, AF.Exp, bias=zero_sb[0:2], scale=-0.5
                )
                it["rmsv"] = rmsv

            def _norm_s2(pool_ps, it):
                nsz = it["nsz"]
                rbc = T(pool_ps, [128, NT], "paux", bufs=3)[:, :nsz]
                nc.tensor.matmul(rbc, it["w"], it["rmsv"], start=True, stop=True)
                nc.vector.tensor_mul(it["dest"], it["tb"], rbc)

            def norm_pump(pool_ps, pool_wk, flush=False):
                items = _nq["items"]
                lag1, lag2 = (0, 0) if flush else (1, 2)
                while _nq["s1"] < len(items) - lag1:
                    _norm_s1(pool_ps, pool_wk, items[_nq["s1"]])
                    _nq["s1"] += 1
                while _nq["s2"] < min(_nq["s1"], len(items) - lag2):
                    _norm_s2(pool_ps, items[_nq["s2"]])
                    _nq["s2"] += 1

            def norm_chunk(pool_ps, pool_wk, psum, w_lhsT, dest):
                """dest = psum * rsqrt(mean_d(psum^2)+eps) * w (bias pre-folded)"""
                nsz = psum.shape[-1]
                tb = T(pool_wk, [128, NT], "w")[:, :nsz]
                nc.scalar.copy(tb, psum)
                sq = T(pool_wk, [128, NT], "w", dt=R32)[:, :nsz]
                nc.vector.tensor_mul(sq, tb, tb)
                _nq["items"].append(
                    dict(tb=tb, sq=sq, w=w_lhsT, dest=dest, nsz=nsz, rmsv=None)
                )
                norm_pump(pool_ps, pool_wk)

            def v_proj(pool_ps, src_sb, t, w_sb, b_sb, j):
                """vS[:, j] = (src.T @ wv + bv) directly in [s, d] layout."""
                pv = T(pool_ps, [128, HPC * HD], "pmain", bufs=4)
                for cc in range(8):
                    nc.tensor.matmul(
                        pv,
                        src_sb[:, cc, t * 128 : (t + 1) * 128],
                        w_sb[:, cc, :],
                        start=(cc == 0),
                        stop=False,
                    )
                nc.tensor.matmul(pv, ones1_sb, b_sb, start=False, stop=True)
                nc.vector.tensor_copy(out=vS[:, j, :], in_=pv)

            with contextlib.ExitStack() as _les:
                if loop_iters is not None and only in (None, "p12"):
                    _les.enter_context(tc.For_i(0, loop_iters, 1))

                # ---- phase 1: KV projection of y (kv rows 2048..2303)
                with (
                    tc.tile_pool(name="pp12", bufs=3, space="PSUM") as pp12,
                    tc.tile_pool(name="wk", bufs=12) as wk,
                ):
                    for mc in range(2):  # [k01, k23]
                        ps = T(pp12, [128, NT], "pmain", bufs=4)[:, :M]
                        for cc in range(8):
                            nc.tensor.matmul(
                                ps,
                                wkv_sb[:, cc, mc * 128 : (mc + 1) * 128],
                                yT_sb[:, cc, :],
                                start=(cc == 0),
                                stop=False,
                            )
                        nc.tensor.matmul(
                            ps, bkr_sb[0:1, mc * 128 : (mc + 1) * 128],
                            onesNT_sb[:, :M], start=False, stop=True,
                        )
                        norm_chunk(pp12, wk, ps, wkn_sb, kT[:, mc, N : N + M])
                    for t in range(2):
                        v_proj(pp12, yT_sb, t, wvy_sb, bvy_sb, 16 + t)

                    # ---- phase 2: QKV projection of x
                    with tc.tile_pool(name="xin", bufs=2) as xin:
                        for nt in range(N // NT):
                            nsl = slice(nt * NT, (nt + 1) * NT)
                            xc = T(xin, [128, 8, NT], "xc", dt=BF16)
                            nc.sync.dma_start(
                                xc, xT.rearrange("(o p) f -> p o f", p=128)[:, :, nsl]
                            )
                            for mc in range(4):  # [q01,q23,k01,k23]
                                ps = T(pp12, [128, NT], "pmain", bufs=4)
                                for cc in range(8):
                                    nc.tensor.matmul(
                                        ps,
                                        wqkv_sb[:, cc, mc * 128 : (mc + 1) * 128],
                                        xc[:, cc, :],
                                        start=(cc == 0),
                                        stop=False,
                                    )
                                nc.tensor.matmul(
                                    ps, bqr_sb[0:1, mc * 128 : (mc + 1) * 128],
                                    onesNT_sb, start=False, stop=True,
                                )
                                if mc < 2:
                                    norm_chunk(pp12, wk, ps,
                                               wqn_sb, qT[:, mc, nsl])
                                else:
                                    norm_chunk(pp12, wk, ps,
                                               wkn_sb, kT[:, mc - 2, nsl])
                            for t in range(4):
                                v_proj(pp12, xc, t, wvx_sb, bvx_sb, nt * 4 + t)
                    norm_pump(pp12, wk, flush=True)

                if only != "p12":
                  with contextlib.ExitStack() as _l3:
                    if loop_iters is not None and only == "p3":
                        _l3.enter_context(tc.For_i(0, loop_iters, 1))
                    # ---- phase 3+4: attention + output projection
                    with (
                        tc.tile_pool(name="pa_plA", bufs=2, space="PSUM") as pa_plA,
                        tc.tile_pool(name="pa_plB", bufs=2, space="PSUM") as pa_plB,
                        tc.tile_pool(name="pa_av", bufs=2, space="PSUM") as pa_av,
                        tc.tile_pool(name="pa_aux", bufs=2, space="PSUM") as pa_aux,
                        tc.tile_pool(name="atp", bufs=12) as atp,
                        tc.tile_pool(name="outp", bufs=2) as outp,
                        tc.tile_pool(name="osp", bufs=3) as osp,
                    ):
                        pl_pools = [pa_plA, pa_plB]
                        for nt in range(N // NT):
                            nsl = slice(nt * NT, (nt + 1) * NT)
                            ot = T(outp, [128, 2, NT], "ot", dt=BF16)
                            av_ps = [
                                T(pa_av, [128, NT], "av", bufs=2)
                                for _ in range(2)
                            ]
                            den_ps = T(pa_aux, [128, NT], "aux", bufs=2)
                            nc.vector.memset(den_ps, 0.0)
                            gran = [(jp, hp)
                                    for jp in range(NJP) for hp in range(2)]
                            at_hist = {}
                            for g in range(len(gran) + 2):
                                if g < len(gran):
                                    jp, hp = gran[g]
                                    pls = [
                                        T(pl_pools[idx], [128, 2 * NT], "pl",
                                          bufs=1, dt=F32)
                                        for idx in range(2)
                                    ]
                                    for u in range(2):
                                        for idx in range(2):
                                            prt = slice(64 * idx, 64 * idx + 64)
                                            j = 2 * jp + u
                                            nc.tensor.matmul(
                                                pls[idx][:, u * NT : (u + 1) * NT],
                                                kT[prt, hp, j * 128 : (j + 1) * 128],
                                                qT[prt, hp, nsl],
                                                start=True, stop=True,
                                                tile_position=(64 * idx, 0),
                                            )
                                    ats = []
                                    for idx in range(2):
                                        t_id = 2 * g + idx
                                        if exp == "copy":
                                            at = T(atp, [128, 2 * NT], "at",
                                                   dt=BF16)
                                            nc.vector.tensor_copy(at, pls[idx])
                                        elif exp == "dve" or (
                                            exp == "hybrid" and t_id % 3 == 2
                                        ):
                                            y0 = T(atp, [128, 2 * NT], "at",
                                                   dt=I16)
                                            nc.vector.tensor_scalar(
                                                y0, pls[idx], float(ALPHA),
                                                float(BETA_P), op0=MUL,
                                                op1=ADD,
                                            )
                                            at = y0.bitcast(BF16)
                                        else:
                                            at = T(atp, [128, 2 * NT], "at",
                                                   dt=BF16)
                                            nc.scalar.activation(
                                                at, pls[idx], AF.Exp,
                                                bias=lnk_sb[:], scale=SCALE,
                                            )
                                        ats.append(at)
                                    at_hist[g] = ats
                                if 0 <= g - 1 < len(gran):
                                    jp1, hp1 = gran[g - 1]
                                    for u in range(2):
                                        j = 2 * jp1 + u
                                        usl = slice(u * NT, (u + 1) * NT)
                                        for idx in range(2):
                                            h = 2 * hp1 + idx
                                            nc.tensor.matmul(
                                                av_ps[hp1][64 * idx : 64 * idx + 64, :],
                                                vS[:, j, 64 * h : 64 * h + 64],
                                                at_hist[g - 1][idx][:, usl],
                                                start=(j == 0),
                                                stop=(j == NCH - 1),
                                                tile_position=(0, 64 * idx),
                                            )
                                if 2 <= g and gran[g - 2][1] == 1:
                                    jp2 = gran[g - 2][0]
                                    at4 = at_hist[g - 3] + at_hist[g - 2]
                                    for u in range(2):
                                        j = 2 * jp2 + u
                                        usl = slice(u * NT, (u + 1) * NT)
                                        for h in range(4):
                                            nc.tensor.matmul(
                                                den_ps[32 * h : 32 * h + 1, :],
                                                ones1c_sb,
                                                at4[h][:, usl],
                                                start=False,
                                                stop=(j == NCH - 1 and h == 3),
                                                tile_position=(0, 32 * h),
                                            )
                            den_sb = T(osp, [128, NT], "den", dt=R32)
                            nc.scalar.copy(den_sb, den_ps)
                            for hp in range(2):
                                dbc = T(pl_pools[hp], [128, NT], "pl",
                                        bufs=1, dt=F32)
                                nc.tensor.matmul(
                                    dbc, sel_sb[:, hp, :], den_sb,
                                    start=True, stop=True,
                                )
                                rbc = T(osp, [128, NT], "rbc")
                                nc.vector.reciprocal_approx_fast(rbc, dbc)
                                nc.vector.tensor_mul(
                                    ot[:, hp, :], av_ps[hp], rbc
                                )

                            for oc in range(8):
                                po = T(pl_pools[oc % 2], [128, NT], "pl",
                                       bufs=1, dt=F32)
                                for cc in range(2):
                                    nc.tensor.matmul(
                                        po,
                                        wproj_sb[:, cc, oc * 128 : (oc + 1) * 128],
                                        ot[:, cc, :],
                                        start=(cc == 0), stop=(cc == 1),
                                    )
                                ob = T(osp, [128, NT], "ob", dt=BF16)
                                nc.vector.tensor_copy(ob, po)
                                nc.sync.dma_start(
                                    outT.rearrange("(o p) f -> p o f", p=128)[:, oc, nsl],
                                    ob,
                                )

    _orig = bacc.get_activation_tables

    def _tables(arch):
        t = _orig(arch)
        return {
            name: (set() if name in ("exp_and_others", "natural_log",
                                     "exp_and_friends") else fns)
            for name, fns in t.items()
        }

    bacc.get_activation_tables = _tables
    try:
        nc.compile()
    finally:
        bacc.get_activation_tables = _orig
    return nc


_PROGRAM = None


def _get_program():
    global _PROGRAM
    if _PROGRAM is None:
        _PROGRAM = build_program()
    return _PROGRAM


def _make_in_maps(x, y, qkv_w, qkv_b, kv_w, kv_b, qn_w, kn_w, proj_w, proj_b):
    f = np.float32
    bf = ml_dtypes.bfloat16
    onesb = np.zeros((128, 2), f)
    onesb[0:64, 0] = 1.0
    onesb[64:128, 1] = 1.0
    wqn = np.zeros((2, 128), f)
    wqn[0, 0:64] = qn_w
    wqn[1, 64:128] = qn_w
    wkn = np.zeros((2, 128), f)
    wkn[0, 0:64] = kn_w
    wkn[1, 64:128] = kn_w
    sel2 = np.zeros((128, 2, 128), f)
    for hp in range(2):
        for a in range(2):
            sel2[32 * (2 * hp + a), hp, 64 * a : 64 * a + 64] = 1.0

    in_maps = []
    for core in range(8):
        b, g = divmod(core, 4)
        qs = slice(g * 256, (g + 1) * 256)
        wqkv = np.concatenate([qkv_w[qs], qkv_w[1024:2048][qs]], axis=0)
        bq = np.concatenate([qkv_b[qs], qkv_b[1024:2048][qs]])
        wkv = kv_w[qs]
        bk = kv_b[qs]
        wvx = qkv_w[2048:3072][qs]
        bvxv = qkv_b[2048:3072][qs]
        wvy = kv_w[1024:2048][qs]
        bvyv = kv_b[1024:2048][qs]
        wproj = np.ascontiguousarray(proj_w[:, qs].T, f)  # [256, 1024]
        in_maps.append(
            {
                "xT": np.ascontiguousarray(x[b].T).astype(bf),
                "yT": np.ascontiguousarray(y[b].T).astype(bf),
                "wqkvT": np.ascontiguousarray(wqkv.T).astype(bf),
                "bqr": bq.reshape(1, 512).astype(bf),
                "wkvT": np.ascontiguousarray(wkv.T).astype(bf),
                "bkr": bk.reshape(1, 256).astype(bf),
                "wvxT": np.ascontiguousarray(wvx.T).astype(bf),
                "bvx": bvxv.reshape(1, 256).astype(bf),
                "wvyT": np.ascontiguousarray(wvy.T).astype(bf),
                "bvy": bvyv.reshape(1, 256).astype(bf),
                "ones1r": np.ones((1, 128), bf),
                "wprojT": np.ascontiguousarray(
                    wproj.reshape(2, 128, C).transpose(1, 0, 2)
                ).astype(bf),
                "onesb": onesb,
                "wqn_b": wqn,
                "wkn_b": wkn,
                "ones1c": np.ones((128, 1), bf),
                "sel2": np.ascontiguousarray(sel2.reshape(128, 256)),
            }
        )
    return in_maps


def run_cores(inputs, trace=False, **kwargs):
    nc = _get_program()
    in_maps = _make_in_maps(**{k: np.asarray(v, np.float32) for k, v in inputs.items()})
    return run_bass_kernel_spmd(
        nc, in_maps, core_ids=list(range(8)), trace=trace, **kwargs
    )


def kernel(**inputs):
    proj_b = np.asarray(inputs["proj_b"], np.float32)
    res = run_cores(inputs).results
    out = np.zeros((B, N, C), np.float32)
    for core in range(8):
        b = core // 4
        out[b] += np.asarray(res[core]["outT"], np.float32).T
    out += proj_b[None, None, :]
    return out
